# revision 1
# baseline (speedup 1.0000x reference)
"""CavityLoss Trainium2 kernel (nn_CavityLoss_43722767073667).

Mathematical reduction of the reference, exact in fp32 (verified against a
bit-faithful numpy emulation incl. adversarial threshold-boundary values):

  pb = (floor(pred*255) >= 128)  <=>  (pred >= c*),  c* = f32(128/255)
  The 5^3 all-ones dilation of the binary gt is an exact integer count
  >= gt (the window contains the center voxel), so
      diff = ((gt - pb*dilate(gt)) > 0) == gt * (1 - pb)     [identity]
  Non-critical voxels contribute exactly 0 to the BCE in fp32:
      clip(0, 1e-12, 1-1e-12) -> 1e-12, and fp32(1 - 1e-12) == 1.0,
      so (1-lc)*log(1-pc_c) == log(1.0) == 0.
  Therefore  loss = -mean( gt * [pred < c*] * ln(pred) ).

Distribution: 192^3 volume flattened and split into 8 equal slabs (depth
sharding: 24 z-planes per core), each viewed as [128 partitions, 6912].
Pointwise + reduction only - the dilation cancels, so no halo exchange and
no collectives; the cross-core mean is combined on the host in f64.

Per-core device kernel (raw bacc, hand-rolled semaphores, no Tile):
  sync engine streams pred/gt tiles in on the qSP HWDGE ring
  DVE   STT#1: r = (p is_ge c*) max p        # r = p where p<c*, else 1.0
  ACT   Ln:    l = ln(r)                     # masked ln; ln(1) ~ 0
  DVE   STT#2: (l bypass 1) mult gt, accum_out -> per-partition row sums
  PE    ones^T @ acc                         # 128-partition reduce -> [1,NT]
  sync  one contiguous 20-byte DMA of the [1,NT] result

Scheduling notes (measured on HW):
  - one semaphore per DMA transfer (completion order across queues is not
    FIFO, a shared counter would race - caught by CoreSim)
  - exactly one wait per instruction (TRN2 HW limit; gt arrival is proxied
    through ACT's wait so DVE's STT#2 only waits on s_l)
  - DVE stream is software-pipelined (STT#1(t+1) before STT#2(t)) so the
    serial STT#1->Ln->STT#2 chain spans tiles instead of serializing
  - a dummy Ln on the const-1.0 tile hoists the ~2.7us ACT_TABLE_LOAD
    into the DMA wait window
  - progressive tile sizes: the last tile is small so the post-last-byte
    compute tail (Ln + STT#2 of the final tile) is short
"""

import numpy as np

import concourse.bacc as bacc
import concourse.mybir as mybir
from concourse.bass_utils import run_bass_kernel_spmd

D = 192
N_CORES = 8
P = 128
TOTAL = D * D * D              # 7_077_888
PER_CORE = TOTAL // N_CORES    # 884_736
FREE = PER_CORE // P           # 6_912
SIZES = [1728, 1728, 1728, 1152, 576]
assert sum(SIZES) == FREE
NT = len(SIZES)

C_STAR = float(np.float32(128.0) / np.float32(255.0))

_CACHE = {}


def _build():
    nc = bacc.Bacc("TRN2", name="cavity_loss")
    f32 = mybir.dt.float32
    pred = nc.dram_tensor("pred", [P, FREE], f32, kind="ExternalInput")
    gt = nc.dram_tensor("gt", [P, FREE], f32, kind="ExternalInput")
    out = nc.dram_tensor("out", [1, NT], f32, kind="ExternalOutput")

    ge = mybir.AluOpType.is_ge
    mx = mybir.AluOpType.max
    byp = mybir.AluOpType.bypass
    mul = mybir.AluOpType.mult
    Ln = mybir.ActivationFunctionType.Ln

    pred_sb = nc.alloc_sbuf_tensor("pred_sb", [P, FREE], f32).ap()
    gt_sb = nc.alloc_sbuf_tensor("gt_sb", [P, FREE], f32).ap()
    r_sb = nc.alloc_sbuf_tensor("r_sb", [P, FREE], f32).ap()
    l_sb = nc.alloc_sbuf_tensor("l_sb", [P, FREE], f32).ap()
    acc = nc.alloc_sbuf_tensor("acc_sb", [P, NT], f32).ap()

    s_pred = [nc.alloc_semaphore(f"s_pred{t}") for t in range(NT)]
    s_gt = [nc.alloc_semaphore(f"s_gt{t}") for t in range(NT)]
    s_r = nc.alloc_semaphore("s_r")
    s_l = nc.alloc_semaphore("s_l")
    s_acc = nc.alloc_semaphore("s_acc")
    s_mm = nc.alloc_semaphore("s_mm")
    s_fin = nc.alloc_semaphore("s_fin")
    s_out = nc.alloc_semaphore("s_out")

    offs = np.concatenate([[0], np.cumsum(SIZES)]).tolist()
    sls = [slice(offs[t], offs[t + 1]) for t in range(NT)]

    # sync: stream all tiles in on one HWDGE ring, pred before gt per tile
    for t in range(NT):
        nc.sync.dma_start(pred_sb[:, sls[t]], pred[:, sls[t]]).then_inc(s_pred[t], 16)
        nc.sync.dma_start(gt_sb[:, sls[t]], gt[:, sls[t]]).then_inc(s_gt[t], 16)

    # scalar: dummy Ln pulls ACT_TABLE_LOAD into the DMA window, then the
    # per-tile Ln chain (gt arrival proxied so STT#2 needs a single wait)
    dummy = nc.alloc_sbuf_tensor("dummy_sb", [P, 1], f32).ap()
    nc.scalar.activation(dummy[:], nc.const_aps.tensor(1.0, (P, 1)), Ln)
    for t in range(NT):
        sl = sls[t]
        nc.scalar.wait_ge(s_gt[t], 16)
        nc.scalar.wait_ge(s_r, t + 1)
        nc.scalar.activation(l_sb[:, sl], r_sb[:, sl], Ln).then_inc(s_l, 1)

    # vector, software-pipelined across tiles
    def stt1(t):
        sl = sls[t]
        nc.vector.wait_ge(s_pred[t], 16)
        nc.vector.scalar_tensor_tensor(
            r_sb[:, sl], pred_sb[:, sl], C_STAR, pred_sb[:, sl], ge, mx
        ).then_inc(s_r, 1)

    def stt2(t):
        sl = sls[t]
        nc.vector.wait_ge(s_l, t + 1)
        # out lands over r_sb tile t: dead after Ln(t), ordered via s_l wait
        nc.vector.scalar_tensor_tensor(
            r_sb[:, sl], l_sb[:, sl], 1.0, gt_sb[:, sl], byp, mul,
            accum_out=acc[:, t : t + 1],
        ).then_inc(s_acc, 1)

    stt1(0)
    for t in range(1, NT):
        stt1(t)
        stt2(t - 1)
    stt2(NT - 1)

    # finalize: partition-reduce acc on the (otherwise idle) TensorEngine,
    # then one contiguous tiny DMA: [1, NT] on one partition = 1 descriptor
    psum_fin = nc.alloc_psum_tensor("psum_fin", [1, NT], f32).ap()
    fin_sb = nc.alloc_sbuf_tensor("fin_sb", [1, NT], f32).ap()
    ones = nc.const_aps.tensor(1.0, (P, 1))
    nc.tensor.wait_ge(s_acc, NT)
    nc.tensor.matmul(
        psum_fin[:], ones, acc[:], start=True, stop=True
    ).then_inc(s_mm, 1)
    nc.vector.wait_ge(s_mm, 1)
    nc.vector.tensor_copy(fin_sb[:], psum_fin[:]).then_inc(s_fin, 1)
    nc.sync.wait_ge(s_fin, 1)
    nc.sync.dma_start(out[:], fin_sb[:]).then_inc(s_out, 16)
    nc.sync.wait_ge(s_out, 16)

    nc.compile()
    return nc


def _get_nc():
    if "nc" not in _CACHE:
        _CACHE["nc"] = _build()
    return _CACHE["nc"]


def _shard(x):
    flat = np.ascontiguousarray(np.asarray(x, dtype=np.float32)).reshape(-1)
    assert flat.size == TOTAL, f"expected {TOTAL} elements, got {flat.size}"
    return [
        flat[c * PER_CORE : (c + 1) * PER_CORE].reshape(P, FREE)
        for c in range(N_CORES)
    ]


def run_spmd(pred, gt, **kw):
    """Shard, run on 8 cores; returns BassKernelResults (kw e.g. trace=True)."""
    preds = _shard(pred)
    gts = _shard(gt)
    in_maps = [{"pred": preds[c], "gt": gts[c]} for c in range(N_CORES)]
    return run_bass_kernel_spmd(
        _get_nc(), in_maps, core_ids=list(range(N_CORES)), **kw
    )


def kernel(pred, gt):
    res = run_spmd(pred, gt)
    total = 0.0
    for r in res.results:
        total += float(r["out"].astype(np.float64).sum())
    return np.asarray(np.float32(-total / TOTAL))



# revision 2
# speedup vs baseline: 1.1709x; 1.1709x over previous
"""CavityLoss Trainium2 kernel (nn_CavityLoss_43722767073667), v2.

Mathematical reduction of the reference, exact in fp32 (verified, incl. the
dilation identity and the clip/log algebra — see the v1 docstring history):

    loss = -mean( gt * [pred < c*] * ln(pred) ),  c* = f32(128/255)

v2 over v1 (35.2us): the trace showed the v1 timeline was (a) 17.5us of DMA
streaming 7.08 MB/core of fp32, (b) a ~7us serial fp32-DVE compute tail.
Fixes here:
  - fp16 transport: pred as fp16 (rel err ~2e-4 on the uniform input, vs the
    2e-2 gate; fp16 is strictly better than bf16 for (0,1) values), gt is
    binary so fp16 is lossless. Halves HBM traffic to 3.54 MB/core.
  - pred+gt packed per tile into ONE dram tensor -> one DMA (and one
    semaphore) per tile instead of two.
  - all DVE ops in fp16 -> 2x_1p perf mode (2 elem/cycle/lane).
  - Ln runs on RAW pred (mask applied after, 0*ln kills excluded terms), so
    ACT depends only on the DMA, not on DVE -> shorter critical chain.
  - the [128, NT] fp32 row-sum accumulators are DMA'd straight out and
    reduced on host in f64 (drops the PE matmul + PSUM copy of v1).

Per-core device program:
  sync  one packed DMA per tile on the qSP HWDGE ring
  ACT   Ln:    l = ln(p)                  (fp16 out; dummy Ln hoists the
                                           ~2.7us ACT_TABLE_LOAD into the
                                           DMA window)
  DVE   STT-A: q = (p is_lt c*) mult gt   # critical mask, {0,1} in fp16
  DVE   STT-B: (q bypass 1) mult l, accum_out -> acc[:, t]  # row sums
  sync  one [128, NT] f32 DMA of acc

Scheduling: one sem per DMA; every instruction has exactly one wait (TRN2
limit): STT-A(t) waits the tile DMA, Ln(t) waits the tile DMA, STT-B(t)
waits s_l>=t+1 (A(t) is earlier in DVE program order). The DVE stream is
software-pipelined (A(t+1) issued before B(t)). Progressive tile sizes keep
the post-last-byte tail short.
"""

import numpy as np

import concourse.bacc as bacc
import concourse.mybir as mybir
from concourse.bass_utils import run_bass_kernel_spmd

D = 192
N_CORES = 8
P = 128
TOTAL = D * D * D              # 7_077_888
PER_CORE = TOTAL // N_CORES    # 884_736
FREE = PER_CORE // P           # 6_912
SIZES = [1024, 1536, 1536, 1536, 896, 384]
assert sum(SIZES) == FREE
assert all(s % 2 == 0 for s in SIZES)
NT = len(SIZES)
OFFS = np.concatenate([[0], np.cumsum(SIZES)]).tolist()

C_STAR = float(np.float32(128.0) / np.float32(255.0))

_CACHE = {}


def _build():
    nc = bacc.Bacc("TRN2", name="cavity_loss")
    f32 = mybir.dt.float32
    f16 = mybir.dt.float16
    inp = nc.dram_tensor("inp", [P, 2 * FREE], f16, kind="ExternalInput")
    out = nc.dram_tensor("out", [P, NT], f32, kind="ExternalOutput")

    lt = mybir.AluOpType.is_lt
    byp = mybir.AluOpType.bypass
    mul = mybir.AluOpType.mult
    Ln = mybir.ActivationFunctionType.Ln

    in_sb = nc.alloc_sbuf_tensor("in_sb", [P, 2 * FREE], f16).ap()
    q_sb = nc.alloc_sbuf_tensor("q_sb", [P, FREE], f16).ap()
    l_sb = nc.alloc_sbuf_tensor("l_sb", [P, FREE], f16).ap()
    acc = nc.alloc_sbuf_tensor("acc_sb", [P, NT], f32).ap()

    s_in = [nc.alloc_semaphore(f"s_in{t}") for t in range(NT)]
    s_l = nc.alloc_semaphore("s_l")
    s_acc = nc.alloc_semaphore("s_acc")
    s_out = nc.alloc_semaphore("s_out")

    # packed layout: tile t occupies cols [2*o, 2*o + 2*s) of inp/in_sb,
    # pred in the first s cols, gt in the next s.
    def pr(t):
        o, s = OFFS[t], SIZES[t]
        return in_sb[:, 2 * o : 2 * o + s]

    def gr(t):
        o, s = OFFS[t], SIZES[t]
        return in_sb[:, 2 * o + s : 2 * o + 2 * s]

    def sl(t):
        return slice(OFFS[t], OFFS[t + 1])

    # sync: stream the packed tiles in on one HWDGE ring
    for t in range(NT):
        o, s = OFFS[t], SIZES[t]
        nc.sync.dma_start(
            in_sb[:, 2 * o : 2 * o + 2 * s], inp[:, 2 * o : 2 * o + 2 * s]
        ).then_inc(s_in[t], 16)

    # scalar: dummy Ln pulls ACT_TABLE_LOAD into the DMA window, then per-tile
    # Ln on raw pred (mask applied later; 0*ln kills excluded terms)
    dummy = nc.alloc_sbuf_tensor("dummy_sb", [P, 1], f32).ap()
    nc.scalar.activation(dummy[:], nc.const_aps.tensor(1.0, (P, 1)), Ln)
    for t in range(NT):
        nc.scalar.wait_ge(s_in[t], 16)
        nc.scalar.activation(l_sb[:, sl(t)], pr(t), Ln).then_inc(s_l, 1)

    # vector, software-pipelined across tiles; all operands fp16 -> 2x mode
    def stt_a(t):
        nc.vector.wait_ge(s_in[t], 16)
        nc.vector.scalar_tensor_tensor(
            q_sb[:, sl(t)], pr(t), C_STAR, gr(t), lt, mul
        )

    def stt_b(t):
        nc.vector.wait_ge(s_l, t + 1)
        # out lands back over q tile t (dead after this op)
        nc.vector.scalar_tensor_tensor(
            q_sb[:, sl(t)], q_sb[:, sl(t)], 1.0, l_sb[:, sl(t)], byp, mul,
            accum_out=acc[:, t : t + 1],
        ).then_inc(s_acc, 1)

    stt_a(0)
    for t in range(1, NT):
        stt_a(t)
        stt_b(t - 1)
    stt_b(NT - 1)

    # finalize: one contiguous [128, NT] f32 DMA; host reduces in f64
    nc.sync.wait_ge(s_acc, NT)
    nc.sync.dma_start(out[:], acc[:]).then_inc(s_out, 16)
    nc.sync.wait_ge(s_out, 16)

    nc.compile()
    return nc


def _get_nc():
    if "nc" not in _CACHE:
        _CACHE["nc"] = _build()
    return _CACHE["nc"]


def _pack(pred, gt):
    p = np.ascontiguousarray(np.asarray(pred, dtype=np.float32)).reshape(-1)
    g = np.ascontiguousarray(np.asarray(gt, dtype=np.float32)).reshape(-1)
    assert p.size == TOTAL and g.size == TOTAL
    p16 = p.astype(np.float16).reshape(N_CORES, P, FREE)
    g16 = g.astype(np.float16).reshape(N_CORES, P, FREE)
    packed = np.empty((N_CORES, P, 2 * FREE), np.float16)
    for t in range(NT):
        o, s = OFFS[t], SIZES[t]
        packed[:, :, 2 * o : 2 * o + s] = p16[:, :, o : o + s]
        packed[:, :, 2 * o + s : 2 * o + 2 * s] = g16[:, :, o : o + s]
    return packed


def run_spmd(pred, gt, **kw):
    """Shard, run on 8 cores; returns BassKernelResults (kw e.g. trace=True)."""
    packed = _pack(pred, gt)
    in_maps = [{"inp": packed[c]} for c in range(N_CORES)]
    return run_bass_kernel_spmd(
        _get_nc(), in_maps, core_ids=list(range(N_CORES)), **kw
    )


def kernel(pred, gt):
    res = run_spmd(pred, gt)
    total = 0.0
    for r in res.results:
        total += float(r["out"].astype(np.float64).sum())
    return np.asarray(np.float32(-total / TOTAL))


# revision 8
# speedup vs baseline: 1.3971x; 1.1932x over previous
"""CavityLoss Trainium2 kernel (nn_CavityLoss_43722767073667), v4.

Mathematical reduction of the reference, exact in fp32 (verified):

    loss = -mean( gt * [pred < c*] * ln(pred) ),  c* = f32(128/255)

History: v1 35.2us (fp32, STT pipeline) -> v2 29.8us (fp16 transport; trace
showed STT has no fast-mode uop, DVE 16us spine) -> v3 (min/max algebra on
fast ops, but tensor_scalar+accum lowers to TENSOR_SCALAR_CACHE_REDUCE which
runs 1x -> DVE reductions are all slow) -> v4: NO reductions on DVE at all.

Per-element algebra on the fp16 grid (c16 = fp16(c*) = 0.501953125,
rel err vs f32 threshold semantics ~2e-4, gate is 2e-2):

    g_enc = (1-gt)*c16          host-side lossless recode of binary gt
    w   = max(p, g_enc)         DVE tensor_tensor   2x_1p (0.5 cyc/elem)
    z   = min(w, c16)           DVE tensor_scalar   4x_2p (0.25)
    ind = [w < c16]             DVE tensor_scalar   4x_2p (0.25), bf16
    s   = Ln(z * (1/c16))       ACT 1x, accum_out -> row sums (the only
                                free reduction in the machine)
        = ln p - ln c16  on critical voxels (gt=1 and p<c16)
        = ln(1.0f) = 0   exactly, on all excluded voxels
    N1  = sum(ind)              PE: 54 accumulating ind[:,c:c+128]^T @ ones
                                matmuls into one PSUM [128,1] (PE is idle)
    loss = -(sum(s) + N1*ln(c16)) / N        host, f64

Engine budget per core (884736 elems): DMA 3.54 MB ~8.7us, DVE ~9.0us,
ACT ~7.5us, PE ~6us fully overlapped. DVE order is w,z,ind per tile so
ACT's Ln(t) (gated by z via s_z) starts two ops after the tile lands.

Scheduling: one packed pred|g_enc DMA + one sem per tile; every instruction
has exactly one wait (TRN2 limit; consecutive standalone wait_ge
instructions are used where two conditions gate one op). Final accumulators
([128, NT] Ln row sums + [128,1] PSUM counts copied by DVE) leave in one
[128, NT+1] f32 DMA; host reduces in f64.
"""

import numpy as np

import concourse.bacc as bacc
import concourse.mybir as mybir
from concourse.bass_utils import run_bass_kernel_spmd

D = 192
N_CORES = 8
P = 128
TOTAL = D * D * D              # 7_077_888
PER_CORE = TOTAL // N_CORES    # 884_736
FREE = PER_CORE // P           # 6_912
SIZES = [512, 1792, 1792, 1664, 896, 256]
assert sum(SIZES) == FREE
assert all(s % 128 == 0 for s in SIZES)
NT = len(SIZES)
OFFS = np.concatenate([[0], np.cumsum(SIZES)]).tolist()

C_STAR = np.float32(128.0) / np.float32(255.0)
C16 = float(np.float16(C_STAR))                    # 0.501953125, fp16-exact
INV = float(np.float32(1.0) / np.float32(C16))     # f32(C16)*f32(INV) == 1.0f
LN_C16 = float(np.log(np.float64(C16)))

_CACHE = {}


def _build():
    nc = bacc.Bacc("TRN2", name="cavity_loss")
    f32 = mybir.dt.float32
    f16 = mybir.dt.float16
    bf16 = mybir.dt.bfloat16
    inp = nc.dram_tensor("inp", [P, 2 * FREE], f16, kind="ExternalInput")
    out = nc.dram_tensor("out", [P, NT + 1], f32, kind="ExternalOutput")

    mx = mybir.AluOpType.max
    mn = mybir.AluOpType.min
    lt = mybir.AluOpType.is_lt
    Ln = mybir.ActivationFunctionType.Ln

    in_sb = nc.alloc_sbuf_tensor("in_sb", [P, 2 * FREE], f16).ap()
    w_sb = nc.alloc_sbuf_tensor("w_sb", [P, FREE], f16).ap()
    z_sb = nc.alloc_sbuf_tensor("z_sb", [P, FREE], f16).ap()
    l_sb = nc.alloc_sbuf_tensor("l_sb", [P, FREE], f16).ap()
    ind_sb = nc.alloc_sbuf_tensor("ind_sb", [P, FREE], bf16).ap()
    # cols 0..NT-1: ACT Ln row sums; col NT: PE counts (copied from PSUM)
    acc = nc.alloc_sbuf_tensor("acc_sb", [P, NT + 1], f32).ap()
    psum_n = nc.alloc_psum_tensor("psum_n", [P, 1], f32).ap()

    s_in = [nc.alloc_semaphore(f"s_in{t}") for t in range(NT)]
    s_z = nc.alloc_semaphore("s_z")
    s_ind = nc.alloc_semaphore("s_ind")
    s_acc = nc.alloc_semaphore("s_acc")
    s_mm = nc.alloc_semaphore("s_mm")
    s_cnt = nc.alloc_semaphore("s_cnt")
    s_out = nc.alloc_semaphore("s_out")

    # packed layout: tile t occupies cols [2o, 2o+2s) of inp/in_sb,
    # pred in the first s cols, g_enc in the next s
    def pr(t):
        o, s = OFFS[t], SIZES[t]
        return in_sb[:, 2 * o : 2 * o + s]

    def gr(t):
        o, s = OFFS[t], SIZES[t]
        return in_sb[:, 2 * o + s : 2 * o + 2 * s]

    def sl(t):
        return slice(OFFS[t], OFFS[t + 1])

    # sync: stream the packed tiles in on one HWDGE ring
    for t in range(NT):
        o, s = OFFS[t], SIZES[t]
        nc.sync.dma_start(
            in_sb[:, 2 * o : 2 * o + 2 * s], inp[:, 2 * o : 2 * o + 2 * s]
        ).then_inc(s_in[t], 16)

    # scalar: dummy Ln pulls the ~2.7us ACT_TABLE_LOAD into the DMA window,
    # then per-tile masked-log with row-sum accumulation
    dummy = nc.alloc_sbuf_tensor("dummy_sb", [P, 1], f32).ap()
    nc.scalar.activation(dummy[:], nc.const_aps.tensor(1.0, (P, 1)), Ln)
    for t in range(NT):
        nc.scalar.wait_ge(s_z, t + 1)
        nc.scalar.activation(
            l_sb[:, sl(t)], z_sb[:, sl(t)], Ln, scale=INV,
            accum_out=acc[:, t : t + 1],
        ).then_inc(s_acc, 1)

    # vector: w, z, ind per tile — all plain fast-mode ops, no reductions
    for t in range(NT):
        nc.vector.wait_ge(s_in[t], 16)
        nc.vector.tensor_tensor(w_sb[:, sl(t)], pr(t), gr(t), mx)
        nc.vector.tensor_scalar(
            z_sb[:, sl(t)], w_sb[:, sl(t)], C16, None, mn
        ).then_inc(s_z, 1)
        nc.vector.tensor_scalar(
            ind_sb[:, sl(t)], w_sb[:, sl(t)], C16, None, lt
        ).then_inc(s_ind, 1)
    # after all tiles: copy the PE count column out of PSUM
    nc.vector.wait_ge(s_mm, 1)
    nc.vector.tensor_copy(acc[:, NT : NT + 1], psum_n[:]).then_inc(s_cnt, 1)

    # tensor: count critical voxels — accumulate ind^T @ ones chunks in PSUM
    ones16 = nc.const_aps.tensor(1.0, (P, 1), bf16)
    n_chunks = FREE // 128
    ci = 0
    for t in range(NT):
        nc.tensor.wait_ge(s_ind, t + 1)
        o, s = OFFS[t], SIZES[t]
        for c in range(o, o + s, 128):
            mm = nc.tensor.matmul(
                psum_n[:], ind_sb[:, c : c + 128], ones16,
                start=(ci == 0), stop=(ci == n_chunks - 1),
            )
            ci += 1
    mm.then_inc(s_mm, 1)

    # finalize: one contiguous [128, NT+1] f32 DMA; host reduces in f64
    nc.sync.wait_ge(s_acc, NT)
    nc.sync.wait_ge(s_cnt, 1)
    nc.sync.dma_start(out[:], acc[:]).then_inc(s_out, 16)
    nc.sync.wait_ge(s_out, 16)

    nc.compile()
    return nc


def _get_nc():
    if "nc" not in _CACHE:
        _CACHE["nc"] = _build()
    return _CACHE["nc"]


def _pack(pred, gt):
    p = np.ascontiguousarray(np.asarray(pred, dtype=np.float32)).reshape(-1)
    g = np.ascontiguousarray(np.asarray(gt, dtype=np.float32)).reshape(-1)
    assert p.size == TOTAL and g.size == TOTAL
    p16 = p.astype(np.float16).reshape(N_CORES, P, FREE)
    g16 = ((np.float32(1.0) - g) * np.float32(C16)).astype(np.float16)
    g16 = g16.reshape(N_CORES, P, FREE)
    packed = np.empty((N_CORES, P, 2 * FREE), np.float16)
    for t in range(NT):
        o, s = OFFS[t], SIZES[t]
        packed[:, :, 2 * o : 2 * o + s] = p16[:, :, o : o + s]
        packed[:, :, 2 * o + s : 2 * o + 2 * s] = g16[:, :, o : o + s]
    return packed


def run_spmd(pred, gt, **kw):
    """Shard, run on 8 cores; returns BassKernelResults (kw e.g. trace=True)."""
    packed = _pack(pred, gt)
    in_maps = [{"inp": packed[c]} for c in range(N_CORES)]
    return run_bass_kernel_spmd(
        _get_nc(), in_maps, core_ids=list(range(N_CORES)), **kw
    )


def kernel(pred, gt):
    res = run_spmd(pred, gt)
    loss_sum = 0.0
    for r in res.results:
        a = r["out"].astype(np.float64)
        loss_sum += a[:, :NT].sum() + a[:, NT].sum() * LN_C16
    return np.asarray(np.float32(-loss_sum / TOTAL))
